# revision 13
# baseline (speedup 1.0000x reference)
"""Trainium2 Bass kernel for nn_K_attention_ex (gaussian-kernel residual attention).

Reference computation (per batch sample b):
    sq_i   = ||x_i||^2
    G      = x @ x^T                      (T,T) gram
    sqdist = relu(sq_i + sq_j - 2 G)
    K      = exp(-sqdist * r + m) * (1 - eye)
    out    = x + K @ x

Algebraic restructuring used here (exact up to fp rounding):
    K_full = exp(-r*(sq_i + sq_j - 2 g_ij) + m)
           = beta * e_i * e_j * exp(2 r g_ij),   e = exp(-r*sq), beta = exp(m)
    The diagonal of K_full is beta*e_i^2*exp(2 r sq_i) = beta (exactly, in
    exact arithmetic), so zeroing the diagonal is equivalent to subtracting
    beta*x from K_full @ x:
    out = x + K_full @ x - beta*x
        = (1-beta)*x + beta * e ⊙_row ( E @ (e ⊙_row x) ),  E = exp(2 r G)
    (relu is dropped: sqdist >= 0 up to rounding; the exp of the tiny
     negative rounding residue is a ~1e-7 relative perturbation.)

Sharding: data-parallel over batch B=16 across 8 NeuronCores (2 samples each).

Per-core dataflow (per sample):
    x_sb  (128,16,64)  natural layout, partition p = t%128, k = t//128
    xT    (64,2048)    via 16 PE transposes (chunks of 4) + DVE evacuation
    sq -> e -> f=beta*e, xs = e⊙x, ax = alpha*x  (DVE/ACT prep)
    J-loop (16 row blocks of 128):
        G half (128,1024) psum  = 2 matmuls (N=512), lhsT = xT[:,Jblk]
        E half (128,1024) sbuf  = ACT exp(2r * G)   (scale read from sbuf)
        YT (64,2048) psum      += xs[J]^T-stationary @ E   (4 matmuls N=512)
    YT -> sbuf -> 16 PE transposes -> Y natural (psum) -> DVE:
        out = ax + f ⊙_row Y ;  DMA out.
"""

import numpy as np

import concourse.bass as bass
import concourse.tile as tile
from concourse import bacc, mybir
from concourse.bass_utils import run_bass_kernel_spmd
from concourse.masks import make_identity

F32 = mybir.dt.float32
F32R = mybir.dt.float32r  # fp32 data, PE fast-fp32 matmul mode (1 cyc/col @ N>=256)
AF = mybir.ActivationFunctionType
B, T, C = 16, 2048, 64
N_CORES = 8
BPC = B // N_CORES          # samples per core
NK = T // 128               # 16 row-blocks of 128


def build_nc(reps=1, stages='all'):
    nc = bacc.Bacc("TRN2", target_bir_lowering=False, debug=False, num_devices=N_CORES)
    x_in = nc.dram_tensor("x", [BPC, T, C], F32, kind="ExternalInput")
    r_in = nc.dram_tensor("r_sigma", [1], F32, kind="ExternalInput")
    m_in = nc.dram_tensor("margin", [1], F32, kind="ExternalInput")
    o_out = nc.dram_tensor("out", [BPC, T, C], F32, kind="ExternalOutput")

    with tile.TileContext(nc) as tc:
        if reps == 1:
            _body(tc, o_out.ap(), x_in.ap(), r_in.ap(), m_in.ap(), stages)
        else:
            with tc.For_i(0, reps, 1):
                _body(tc, o_out.ap(), x_in.ap(), r_in.ap(), m_in.ap(), stages)
    nc.compile()
    return nc


LEVELS = {'xload': 0, 'xt': 1, 'prep': 2, 'gram': 3, 'exp': 4, 'y': 5, 'all': 6}


def _body(tc, out_ap, x_ap, r_ap, m_ap, stages='all'):
    lvl = LEVELS[stages]
    do = lambda name: lvl >= LEVELS.get(name, 6)
    nc = tc.nc
    with (
        tc.tile_pool(name="consts", bufs=1) as consts,
        tc.tile_pool(name="sx", bufs=2) as sx,
        tc.tile_pool(name="epool", bufs=3) as epool,
        tc.tile_pool(name="psG", bufs=2, space="PSUM") as psG,
        tc.tile_pool(name="psY", bufs=1, space="PSUM") as psY,
    ):
        # ---- one-time constants ----
        ident = consts.tile([128, 128], F32)
        make_identity(nc, ident)
        rb = consts.tile([128, 1], F32)
        nc.gpsimd.dma_start(out=rb, in_=r_ap.to_broadcast((128, 1)))
        mb = consts.tile([128, 1], F32)
        nc.gpsimd.dma_start(out=mb, in_=m_ap.to_broadcast((128, 1)))
        negr = consts.tile([128, 1], F32)
        nc.vector.tensor_scalar_mul(out=negr, in0=rb, scalar1=-1.0)
        s2r = consts.tile([128, 1], F32)
        nc.vector.tensor_scalar_mul(out=s2r, in0=rb, scalar1=2.0)
        beta = consts.tile([128, 1], F32)
        nc.scalar.activation(out=beta, in_=mb, func=AF.Exp)
        alpha = consts.tile([128, 1], F32)  # 1 - beta
        nc.vector.tensor_scalar(
            out=alpha, in0=beta, scalar1=-1.0, scalar2=1.0,
            op0=mybir.AluOpType.mult, op1=mybir.AluOpType.add,
        )

        for s in range(BPC):
            xv = x_ap[s].rearrange("(p k) c -> p k c", p=128)
            x_sb = sx.tile([128, NK, C], F32)
            nc.sync.dma_start(out=x_sb, in_=xv)

            # ---- xT (64, T) via PE transposes ----
            xT = sx.tile([64, T], F32R)
            for g in range(4 if do('xt') else 0):
                xtr = psG.tile([64, 4, 128], F32, tag="G")
                for kk in range(4):
                    k = 4 * g + kk
                    nc.tensor.transpose(
                        out=xtr[:, kk, :], in_=x_sb[:, k, :], identity=ident
                    )
                nc.vector.tensor_copy(
                    out=xT[:, 512 * g : 512 * (g + 1)],
                    in_=xtr.rearrange("p a b -> p (a b)"),
                )

            # ---- per-row scalars ----
            if not do('prep'):
                continue
            xsq = sx.tile([128, NK, C], F32)
            nc.vector.tensor_mul(xsq, x_sb, x_sb)
            sq = sx.tile([128, NK], F32)
            nc.vector.reduce_sum(out=sq, in_=xsq, axis=mybir.AxisListType.X)
            e = sx.tile([128, NK], F32)
            nc.scalar.activation(out=e, in_=sq, func=AF.Exp, scale=negr)
            f = sx.tile([128, NK], F32)
            nc.vector.tensor_scalar_mul(out=f, in0=e, scalar1=beta)
            xs = sx.tile([128, NK, C], F32R)
            for k in range(NK):
                nc.vector.tensor_scalar_mul(
                    out=xs[:, k, :], in0=x_sb[:, k, :], scalar1=e[:, k : k + 1]
                )
            ax = sx.tile([128, NK, C], F32)
            nc.vector.tensor_scalar_mul(out=ax, in0=x_sb, scalar1=alpha)

            # ---- main loop over row-blocks (software-pipelined) ----
            # PE runs its queue in program order, so Y-matmuls (which wait on
            # ACT's exp) must not be emitted between a gram and its ACT
            # consumer. Emit gram(j+1) before Y(j) so PE always has
            # ACT-independent work queued while ACT(j) runs.
            YT = psY.tile([64, T], F32, name='YT') if do('y') else None

            def emit_gram(j):
                lhsT_g = xT[:, 128 * j : 128 * (j + 1)]
                gs = []
                for h in range(2):
                    G = psG.tile([128, 1024], F32, tag="G", name=f"G_{s}_{j}_{h}")
                    for q in range(2):
                        n0 = 1024 * h + 512 * q
                        nc.tensor.matmul(
                            out=G[:, 512 * q : 512 * (q + 1)],
                            lhsT=lhsT_g,
                            rhs=xT[:, n0 : n0 + 512],
                            start=True,
                            stop=True,
                        )
                    gs.append(G)
                return gs

            def emit_exp(gs, j):
                es = []
                for h in range(2):
                    E = epool.tile([128, 1024], F32R, tag="E", name=f"E_{s}_{j}_{h}")
                    nc.scalar.activation(out=E, in_=gs[h], func=AF.Exp, scale=s2r)
                    es.append(E)
                return es

            def emit_y(es, j):
                for h in range(2):
                    for q in range(2):
                        n0 = 1024 * h + 512 * q
                        nc.tensor.matmul(
                            out=YT[:, n0 : n0 + 512],
                            lhsT=xs[:, j, :],
                            rhs=es[h][:, 512 * q : 512 * (q + 1)],
                            start=(j == 0),
                            stop=(j == NK - 1),
                        )

            if do('gram'):
                def dummy_e(j):
                    es = []
                    for h in range(2):
                        Ez = epool.tile([128, 1024], F32R, tag="E", name=f"Ez_{s}_{j}_{h}")
                        nc.vector.memset(Ez, 1.0)
                        es.append(Ez)
                    return es
                gs = emit_gram(0)
                for j in range(NK):
                    es = emit_exp(gs, j) if do('exp') else dummy_e(j)
                    if j + 1 < NK:
                        gs = emit_gram(j + 1)
                    if do('y'):
                        emit_y(es, j)

            # ---- back to natural layout + combine ----
            if not do('all'):
                continue
            YTsb = sx.tile([64, T], F32)
            if do('y'):
                nc.vector.tensor_copy(out=YTsb, in_=YT)
            else:
                nc.vector.memset(YTsb, 0.0)
            outsb = sx.tile([128, NK, C], F32)
            for g in range(4):
                ytr = psG.tile([128, 4, C], F32, tag="G")
                for kk in range(4):
                    k = 4 * g + kk
                    nc.tensor.transpose(
                        out=ytr[:, kk, :],
                        in_=YTsb[:, 128 * k : 128 * (k + 1)],
                        identity=ident[:64, :64],
                    )
                for kk in range(4):
                    k = 4 * g + kk
                    nc.vector.tensor_scalar_mul(
                        out=outsb[:, k, :], in0=ytr[:, kk, :], scalar1=f[:, k : k + 1]
                    )
            nc.vector.tensor_add(outsb, outsb, ax)
            nc.sync.dma_start(
                out=out_ap[s].rearrange("(p k) c -> p k c", p=128), in_=outsb
            )


_NC_CACHE = {}


def _get_nc(reps=1, stages='all'):
    key = (reps, stages)
    if key not in _NC_CACHE:
        _NC_CACHE[key] = build_nc(reps, stages)
    return _NC_CACHE[key]


def _run(x, r_sigma, margin, trace=False, reps=1, stages='all'):
    nc = _get_nc(reps, stages)
    x = np.ascontiguousarray(np.asarray(x, dtype=np.float32))
    r_sigma = np.ascontiguousarray(np.asarray(r_sigma, dtype=np.float32))
    margin = np.ascontiguousarray(np.asarray(margin, dtype=np.float32))
    in_maps = [
        {
            "x": np.ascontiguousarray(x[c * BPC : (c + 1) * BPC]),
            "r_sigma": r_sigma,
            "margin": margin,
        }
        for c in range(N_CORES)
    ]
    res = run_bass_kernel_spmd(nc, in_maps, core_ids=list(range(N_CORES)), trace=trace)
    out = np.concatenate([res.results[c]["out"] for c in range(N_CORES)], axis=0)
    return out, res


def kernel(x, r_sigma, margin):
    out, _ = _run(x, r_sigma, margin, trace=False)
    return out
